# revision 7
# baseline (speedup 1.0000x reference)
"""Trainium2 Bass kernel for the adaptive-attention LSTM decoder.

Sharding: data-parallel over batch (16 rows per core on 8 cores), weights
replicated.  All recurrent math is feature-major ([features->partitions,
batch->free]) with weight-stationary bf16 matmuls accumulating in f32 PSUM.
"""

import os
from contextlib import ExitStack

import ml_dtypes
import numpy as np

import concourse.bass as bass
import concourse.bacc as bacc
import concourse.tile as tile
from concourse import mybir
from concourse.bass import IndirectOffsetOnAxis, ds, ts
from concourse.bass_utils import run_bass_kernel_spmd
from concourse.masks import make_identity

F32 = mybir.dt.float32
BF = mybir.dt.bfloat16
I32 = mybir.dt.int32
bfnp = ml_dtypes.bfloat16

B, P, D, V, T = 128, 49, 512, 10000, 50
NCORES = 8
BC = B // NCORES  # 16 batch rows per core
PP = P + 1        # 50 attention slots (49 spatial + sentinel)
NS_FULL = T - 1   # 49 decode steps
KC = D // 128     # 4 k-chunks per 512 features
NV, VCH = 20, 500  # vocab split: 20 chunks of 500
SG = 7            # steps per fc output group (49 = 7*7)


def _tile_w(w_t: np.ndarray) -> np.ndarray:
    """[K, M] (already transposed W.T) -> [128, K/128, M/128, 128] bf16."""
    K, M = w_t.shape
    kc, mc = K // 128, M // 128
    return np.ascontiguousarray(
        w_t.reshape(kc, 128, mc, 128).transpose(1, 0, 2, 3)
    ).astype(bfnp)


def _col_bias(b: np.ndarray) -> np.ndarray:
    """[M] f32 -> [128, M/128] with column m = b[128m:128(m+1)]."""
    return np.ascontiguousarray(b.reshape(-1, 128).T).astype(np.float32)


def build_program(ns: int):
    nc = bacc.Bacc("TRN2", target_bir_lowering=False, debug=False)
    NR = ns * BC              # (step, batch) rows per core
    NJ = (NR + 127) // 128    # gather blocks of 128 rows
    groups = [(s, min(SG, ns - s)) for s in range(0, ns, SG)]

    def din(name, shape, dt):
        return nc.dram_tensor(name, shape, dt, kind="ExternalInput").ap()

    embd = din("emb", [V, D], BF)
    idxd = din("idx", [128, NJ], I32)
    spd = din("spT", [128, KC, BC, P], BF)
    gid = din("giT", [128, KC, BC], BF)
    w1xd = din("W1xT", [128, 8, 16, 128], BF)
    wsxd = din("WsxT", [128, 8, 4, 128], BF)
    wvd = din("WvT", [128, 4, 4, 128], BF)
    u1d = din("U1T", [128, 4, 16, 128], BF)
    wh1d = din("Whh1T", [128, 4, 16, 128], BF)
    usd = din("UsT", [128, 4, 4, 128], BF)
    swhd = din("SwhT", [128, 4, 4, 128], BF)
    affsd = din("AffST", [128, 4, 4, 128], BF)
    affhd = din("AffHT", [128, 4, 4, 128], BF)
    wgd = din("WgT", [128, 4, 4, 128], BF)
    wsd = din("WsT2", [128, 4, 4, 128], BF)
    wpd = din("WpT", [128, 4, 4, 128], BF)
    uad = din("UaT", [128, 4, 16, 128], BF)
    uhd = din("Uh1T", [128, 4, 16, 128], BF)
    wh2d = din("Whh2T", [128, 4, 16, 128], BF)
    fcwd = din("FcT", [128, 4, NV, VCH], BF)
    fcbd = din("fcb", [1, NV, VCH], BF)
    whd = din("whv", [128, 4], BF)
    b1d = din("b1", [128, 16], F32)
    b2d = din("b2", [128, 16], F32)
    bsd = din("bs", [128, 4], F32)
    asbd = din("asb", [128, 4], F32)
    ahbd = din("ahb", [128, 4], F32)
    wgbd = din("wgb", [128, 4], F32)
    wsbd = din("wsb", [128, 4], F32)
    wvbd = din("wvb", [128, 4], F32)
    wpbd = din("wpb", [128, 4], F32)
    outd = nc.dram_tensor("out", [NR, V], F32, kind="ExternalOutput").ap()

    with tile.TileContext(nc) as tc, ExitStack() as ctx:
        const = ctx.enter_context(tc.tile_pool(name="const", bufs=1))
        big = ctx.enter_context(tc.tile_pool(name="big", bufs=1))
        st = ctx.enter_context(tc.tile_pool(name="st", bufs=2))
        wk = ctx.enter_context(tc.tile_pool(name="wk", bufs=2))
        ps_g = ctx.enter_context(tc.tile_pool(name="ps_g", bufs=2, space="PSUM"))
        ps_s = ctx.enter_context(tc.tile_pool(name="ps_s", bufs=4, space="PSUM"))
        ps_fc = ctx.enter_context(tc.tile_pool(name="ps_fc", bufs=2, space="PSUM"))

        # ------- resident buffers
        X1sb = big.tile([128, 16, NR], BF)       # W1x @ x_word.T
        Xssb = big.tile([128, 4, NR], BF)        # Wsx @ x_word.T
        csp = big.tile([128, KC, BC, PP], BF)    # spatial.T; slot 49 = s2 (per step)
        vaU = big.tile([128, KC, BC, PP], BF)    # wv@sp.T + wv_b; slot 49 = 0
        H2A = big.tile([128, KC, ns, BC], BF)    # all h2 states (fc lhsT)

        ones = const.tile([1, 128], BF)
        nc.gpsimd.memset(ones[:], 1.0)
        whsb = const.tile([128, 4], BF)
        nc.sync.dma_start(whsb[:], whd[:])
        fcbsb = const.tile([1, NV, VCH], BF)
        nc.sync.dma_start(fcbsb[:], fcbd[:])
        bias_tiles = {}
        for nm, dd in [("b1", b1d), ("b2", b2d), ("bs", bsd), ("asb", asbd),
                       ("ahb", ahbd), ("wgb", wgbd), ("wsb", wsbd),
                       ("wvb", wvbd), ("wpb", wpbd)]:
            bt = const.tile([128, dd.shape[1]], F32, tag=f"bias_{nm}")
            nc.sync.dma_start(bt[:], dd[:])
            bias_tiles[nm] = bt
        b1sb, b2sb, bssb = bias_tiles["b1"], bias_tiles["b2"], bias_tiles["bs"]
        asbsb, ahbsb = bias_tiles["asb"], bias_tiles["ahb"]
        wgbsb, wsbsb = bias_tiles["wgb"], bias_tiles["wsb"]
        wvbsb, wpbsb = bias_tiles["wvb"], bias_tiles["wpb"]

        nc.vector.memzero(vaU[:])

        # ================= PHASE A: gather + transpose + x-projections
        with ExitStack() as actx:
            pha = actx.enter_context(tc.tile_pool(name="pha", bufs=1))
            phw = actx.enter_context(tc.tile_pool(name="phw", bufs=1))

            ident = pha.tile([128, 128], BF)
            make_identity(nc, ident[:])

            idxsb = pha.tile([128, NJ], I32)
            nc.sync.dma_start(idxsb[:], idxd[:])
            embg = pha.tile([128, NJ, D], BF)
            for j in range(NJ):
                nc.gpsimd.indirect_dma_start(
                    out=embg[:, j, :],
                    out_offset=None,
                    in_=embd[:],
                    in_offset=IndirectOffsetOnAxis(ap=idxsb[:, j : j + 1], axis=0),
                )

            # spatial into csp (slots 0..48)
            nc.sync.dma_start(csp[:, :, :, 0:P], spd[:])
            gisb = pha.tile([128, KC, BC], BF)
            nc.sync.dma_start(gisb[:], gid[:])

            # x_word.T  [128, 8, NR]: rows 0-511 = emb.T, 512-1023 = gi.T
            xT = pha.tile([128, 8, NR], BF)
            for k in range(KC):
                for j in range(NJ):
                    pt = ps_s.tile([128, 128], BF, tag="ps")
                    nc.tensor.transpose(
                        out=pt[:], in_=embg[:, j, ts(k, 128)], identity=ident[:]
                    )
                    w = min(128, NR - j * 128)
                    nc.vector.tensor_copy(
                        out=xT[:, k, ds(j * 128, w)], in_=pt[:, :w]
                    )
            for c in range(KC):
                nc.vector.tensor_copy(
                    out=xT[:, 4 + c, :].rearrange("p (t b) -> p t b", b=BC),
                    in_=gisb[:, c : c + 1, :].broadcast_to([128, ns, BC]),
                )

            w1xsb = phw.tile([128, 8, 16, 128], BF)
            nc.sync.dma_start(w1xsb[:], w1xd[:])
            wsxsb = phw.tile([128, 8, 4, 128], BF)
            nc.sync.dma_start(wsxsb[:], wsxd[:])
            wvsb = phw.tile([128, 4, 4, 128], BF)
            nc.sync.dma_start(wvsb[:], wvd[:])

            # X1 = W1x @ xT, Xs = Wsx @ xT   (n-split in halves of NR)
            nh = (NR + 1) // 2
            for name, wsb, xout, mc in (
                ("x1", w1xsb, X1sb, 16),
                ("xs", wsxsb, Xssb, 4),
            ):
                for m in range(mc):
                    for n0 in range(0, NR, nh):
                        nw = min(nh, NR - n0)
                        pp = ps_s.tile([128, nh], F32, tag="ps")
                        for k in range(8):
                            nc.tensor.matmul(
                                pp[:, :nw],
                                wsb[:, k, m, :],
                                xT[:, k, ds(n0, nw)],
                                start=(k == 0),
                                stop=(k == 7),
                            )
                        nc.vector.tensor_copy(
                            out=xout[:, m, ds(n0, nw)], in_=pp[:, :nw]
                        )

            # va = Wv @ sp.T + wv_b  -> vaU slots 0..48  (n-split by b-halves)
            for m in range(KC):
                for h in range(2):
                    pp = ps_s.tile([128, 8 * P], F32, tag="ps")
                    for k in range(KC):
                        nc.tensor.matmul(
                            pp[:],
                            wvsb[:, k, m, :],
                            csp[:, k, ds(8 * h, 8), 0:P],
                            start=(k == 0),
                            stop=(k == KC - 1),
                        )
                    nc.scalar.activation(
                        out=vaU[:, m, ds(8 * h, 8), 0:P],
                        in_=pp[:].rearrange("p (b q) -> p b q", q=P),
                        func=mybir.ActivationFunctionType.Identity,
                        bias=wvbsb[:, m : m + 1],
                    )

        bisect = os.environ.get("KLSTM_BISECT", "full")
        if bisect == "A":
            zt = wk.tile([128, VCH], F32, tag="pf", name="zfill")
            nc.vector.memzero(zt[:])
            for n in range(NV):
                for r0 in range(0, NR, 128):
                    rw = min(128, NR - r0)
                    nc.sync.dma_start(
                        outd[ds(r0, rw), ds(n * VCH, VCH)], zt[:rw, :]
                    )
        # ================= load recurrent weights (pool reuses phase-A space)
        wts = ctx.enter_context(tc.tile_pool(name="wts", bufs=1))
        wtiles = {}
        for nm, dd in [("u1", u1d), ("wh1", wh1d), ("us", usd), ("swh", swhd),
                       ("affs", affsd), ("affh", affhd), ("wg", wgd),
                       ("ws", wsd), ("wp", wpd), ("ua", uad), ("uh", uhd),
                       ("wh2", wh2d)]:
            wt = wts.tile(list(dd.shape), BF, tag=f"w_{nm}", name=f"w_{nm}")
            nc.sync.dma_start(wt[:], dd[:])
            wtiles[nm] = wt

        # ================= initial states
        h1b = st.tile([128, KC, BC], BF, tag="h1")
        h2b = st.tile([128, KC, BC], BF, tag="h2")
        m1 = st.tile([128, KC, BC], F32, tag="m1")
        m2 = st.tile([128, KC, BC], F32, tag="m2")
        for t0 in (h1b, h2b, m1, m2):
            nc.vector.memzero(t0[:])

        AF = mybir.ActivationFunctionType
        OP = mybir.AluOpType

        def gate_act(gs, bias, si, sf, tg, so):
            for m in range(4):
                nc.scalar.activation(si[:, m, :], gs[:, m, :], AF.Sigmoid,
                                     bias=bias[:, m : m + 1])
                nc.scalar.activation(sf[:, m, :], gs[:, 4 + m, :], AF.Sigmoid,
                                     bias=bias[:, 4 + m : 5 + m])
                nc.scalar.activation(tg[:, m, :], gs[:, 8 + m, :], AF.Tanh,
                                     bias=bias[:, 8 + m : 9 + m])
                nc.scalar.activation(so[:, m, :], gs[:, 12 + m, :], AF.Sigmoid,
                                     bias=bias[:, 12 + m : 13 + m])

        # ================= PHASE B: recurrence
        for t in range(ns if bisect != "A" else 0):
            # ---- LSTM1 gates
            G1 = ps_g.tile([128, 16, BC], F32, tag="G")
            for m in range(16):
                mms = [(wtiles["u1"], k, h2b) for k in range(KC)] + [
                    (wtiles["wh1"], k, h1b) for k in range(KC)
                ]
                for i, (wt, k, rhs) in enumerate(mms):
                    nc.tensor.matmul(
                        G1[:, m, :], wt[:, k, m, :], rhs[:, k, :],
                        start=(i == 0), stop=(i == len(mms) - 1),
                    )
            nc.vector.scalar_tensor_tensor(
                out=G1[:], in0=G1[:], scalar=1.0,
                in1=X1sb[:, :, ts(t, BC)], op0=OP.mult, op1=OP.add,
            )
            si = wk.tile([128, KC, BC], F32, tag="si")
            sf = wk.tile([128, KC, BC], F32, tag="sf")
            tg = wk.tile([128, KC, BC], F32, tag="tg")
            so = wk.tile([128, KC, BC], F32, tag="so")
            gate_act(G1, b1sb, si, sf, tg, so)
            nc.vector.tensor_mul(sf[:], sf[:], m1[:])
            nc.vector.tensor_mul(si[:], si[:], tg[:])
            m1n = st.tile([128, KC, BC], F32, tag="m1")
            nc.vector.tensor_add(m1n[:], sf[:], si[:])
            th1 = wk.tile([128, KC, BC], F32, tag="th1")
            nc.scalar.activation(th1[:], m1n[:], AF.Tanh)
            h1n = st.tile([128, KC, BC], BF, tag="h1")
            nc.vector.tensor_mul(h1n[:], so[:], th1[:])

            # ---- visual sentinel s_t
            S = ps_s.tile([128, KC, BC], F32, tag="ps")
            for m in range(KC):
                mms = [(wtiles["us"], k, h2b) for k in range(KC)] + [
                    (wtiles["swh"], k, h1b) for k in range(KC)
                ]
                for i, (wt, k, rhs) in enumerate(mms):
                    nc.tensor.matmul(
                        S[:, m, :], wt[:, k, m, :], rhs[:, k, :],
                        start=(i == 0), stop=(i == len(mms) - 1),
                    )
            nc.vector.scalar_tensor_tensor(
                out=S[:], in0=S[:], scalar=1.0,
                in1=Xssb[:, :, ts(t, BC)], op0=OP.mult, op1=OP.add,
            )
            sgt = wk.tile([128, KC, BC], F32, tag="sgt")
            for m in range(KC):
                nc.scalar.activation(sgt[:, m, :], S[:, m, :], AF.Sigmoid,
                                     bias=bssb[:, m : m + 1])
            s_tb = wk.tile([128, KC, BC], BF, tag="s_tb")
            nc.vector.tensor_mul(s_tb[:], sgt[:], th1[:])

            # ---- s2 = relu(aff_s), ht = tanh(aff_h)
            A2 = ps_s.tile([128, KC, BC], F32, tag="ps")
            HT = ps_s.tile([128, KC, BC], F32, tag="ps")
            for m in range(KC):
                for k in range(KC):
                    nc.tensor.matmul(
                        A2[:, m, :], wtiles["affs"][:, k, m, :], s_tb[:, k, :],
                        start=(k == 0), stop=(k == KC - 1),
                    )
                for k in range(KC):
                    nc.tensor.matmul(
                        HT[:, m, :], wtiles["affh"][:, k, m, :], h1n[:, k, :],
                        start=(k == 0), stop=(k == KC - 1),
                    )
            s2b = wk.tile([128, KC, BC], BF, tag="s2b")
            htb = wk.tile([128, KC, BC], BF, tag="htb")
            for m in range(KC):
                nc.scalar.activation(s2b[:, m, :], A2[:, m, :], AF.Relu,
                                     bias=asbsb[:, m : m + 1])
                nc.scalar.activation(htb[:, m, :], HT[:, m, :], AF.Tanh,
                                     bias=ahbsb[:, m : m + 1])

            # ---- hid = wg@ht + wg_b ; sen = ws@s2 + ws_b
            HID = ps_s.tile([128, KC, BC], F32, tag="ps")
            SEN = ps_s.tile([128, KC, BC], F32, tag="ps")
            for m in range(KC):
                for k in range(KC):
                    nc.tensor.matmul(
                        HID[:, m, :], wtiles["wg"][:, k, m, :], htb[:, k, :],
                        start=(k == 0), stop=(k == KC - 1),
                    )
                for k in range(KC):
                    nc.tensor.matmul(
                        SEN[:, m, :], wtiles["ws"][:, k, m, :], s2b[:, k, :],
                        start=(k == 0), stop=(k == KC - 1),
                    )
            ub = wk.tile([128, KC, BC], BF, tag="ub")
            senb = wk.tile([128, KC, BC], BF, tag="senb")
            for m in range(KC):
                nc.scalar.activation(ub[:, m, :], HID[:, m, :], AF.Identity,
                                     bias=wgbsb[:, m : m + 1])
                nc.scalar.activation(senb[:, m, :], SEN[:, m, :], AF.Identity,
                                     bias=wsbsb[:, m : m + 1])

            # ---- ext = tanh(vaU + u) with slot49 = sen + u; z = wh . ext
            nc.vector.tensor_copy(
                out=vaU[:, :, :, P : P + 1], in_=senb[:].unsqueeze(3)
            )
            zps = [ps_s.tile([1, 8 * PP], F32, tag="ps", name=f"zps{h}")
                   for h in range(2)]
            for c in range(KC):
                ext = wk.tile([128, BC, PP], BF, tag="ef")
                nc.vector.tensor_add(
                    ext[:], vaU[:, c, :, :],
                    ub[:, c, :].unsqueeze(2).broadcast_to([128, BC, PP]),
                )
                nc.scalar.activation(ext[:], ext[:], AF.Tanh)
                for h in range(2):
                    nc.tensor.matmul(
                        zps[h][:], whsb[:, c : c + 1],
                        ext[:, ds(8 * h, 8), :],
                        start=(c == 0), stop=(c == KC - 1),
                    )
            ez = wk.tile([1, BC * PP], BF, tag="ez")
            for h in range(2):
                nc.scalar.activation(ez[:, ds(400 * h, 400)], zps[h][:], AF.Exp)
            den = wk.tile([1, BC], F32, tag="den")
            nc.vector.reduce_sum(
                den[:], ez[:].rearrange("o (b q) -> o b q", q=PP),
                axis=mybir.AxisListType.X,
            )
            rden = wk.tile([1, BC], F32, tag="rden")
            nc.vector.reciprocal(rden[:], den[:])
            alp = wk.tile([1, BC * PP], BF, tag="alp")
            nc.vector.tensor_mul(
                alp[:].rearrange("o (b q) -> o b q", q=PP),
                ez[:].rearrange("o (b q) -> o b q", q=PP),
                rden[:].unsqueeze(2).broadcast_to([1, BC, PP]),
            )
            arep = [ps_s.tile([128, 8 * PP], F32, tag="ps", name=f"arep{h}")
                    for h in range(2)]
            for h in range(2):
                nc.tensor.matmul(
                    arep[h][:], ones[:], alp[:, ds(400 * h, 400)],
                    start=True, stop=True,
                )

            # ---- c_hat (csp slot49 := s2)
            nc.vector.tensor_copy(
                out=csp[:, :, :, P : P + 1], in_=s2b[:].unsqueeze(3)
            )
            craw = wk.tile([128, KC, BC], F32, tag="craw")
            for c in range(KC):
                for h in range(2):
                    prod = wk.tile([128, 8, PP], F32, tag="pf")
                    nc.vector.tensor_mul(
                        prod[:], csp[:, c, ds(8 * h, 8), :],
                        arep[h][:].rearrange("p (b q) -> p b q", q=PP),
                    )
                    nc.vector.reduce_sum(
                        craw[:, c, ds(8 * h, 8)], prod[:],
                        axis=mybir.AxisListType.X,
                    )
            catb = wk.tile([128, KC, BC], BF, tag="catb")
            nc.vector.scalar_tensor_tensor(
                out=catb[:], in0=craw[:], scalar=1.0, in1=htb[:],
                op0=OP.mult, op1=OP.add,
            )

            # ---- att_out = tanh(wp @ (c_hat + ht) + wp_b)
            W = ps_s.tile([128, KC, BC], F32, tag="ps")
            for m in range(KC):
                for k in range(KC):
                    nc.tensor.matmul(
                        W[:, m, :], wtiles["wp"][:, k, m, :], catb[:, k, :],
                        start=(k == 0), stop=(k == KC - 1),
                    )
            attb = wk.tile([128, KC, BC], BF, tag="attb")
            for m in range(KC):
                nc.scalar.activation(attb[:, m, :], W[:, m, :], AF.Tanh,
                                     bias=wpbsb[:, m : m + 1])

            # ---- LSTM2
            G2 = ps_g.tile([128, 16, BC], F32, tag="G")
            for m in range(16):
                mms = ([(wtiles["ua"], k, attb) for k in range(KC)]
                       + [(wtiles["uh"], k, h1n) for k in range(KC)]
                       + [(wtiles["wh2"], k, h2b) for k in range(KC)])
                for i, (wt, k, rhs) in enumerate(mms):
                    nc.tensor.matmul(
                        G2[:, m, :], wt[:, k, m, :], rhs[:, k, :],
                        start=(i == 0), stop=(i == len(mms) - 1),
                    )
            si2 = wk.tile([128, KC, BC], F32, tag="si")
            sf2 = wk.tile([128, KC, BC], F32, tag="sf")
            tg2 = wk.tile([128, KC, BC], F32, tag="tg")
            so2 = wk.tile([128, KC, BC], F32, tag="so")
            gate_act(G2, b2sb, si2, sf2, tg2, so2)
            nc.vector.tensor_mul(sf2[:], sf2[:], m2[:])
            nc.vector.tensor_mul(si2[:], si2[:], tg2[:])
            m2n = st.tile([128, KC, BC], F32, tag="m2")
            nc.vector.tensor_add(m2n[:], sf2[:], si2[:])
            th2 = wk.tile([128, KC, BC], F32, tag="th1")
            nc.scalar.activation(th2[:], m2n[:], AF.Tanh)
            h2n = H2A[:, :, t, :]
            nc.vector.tensor_mul(h2n, so2[:], th2[:])

            h1b, h2b, m1, m2 = h1n, st_alias(H2A, t), m1n, m2n

        # ================= PHASE C: fc projection
        if bisect == "AL":
            zt = wk.tile([128, VCH], F32, tag="pf", name="zfill2")
            nc.vector.memzero(zt[:])
            for n in range(NV):
                for r0 in range(0, NR, 128):
                    rw = min(128, NR - r0)
                    nc.sync.dma_start(
                        outd[ds(r0, rw), ds(n * VCH, VCH)], zt[:rw, :]
                    )
        for n in range(NV if bisect == "full" else 0):
            fcw = wk.tile([128, KC, VCH], BF, tag="ef")
            nc.sync.dma_start(fcw[:], fcwd[:, :, n, :])
            for gi_, (s0, slen) in enumerate(groups):
                rows = slen * BC
                fps = ps_fc.tile([128, VCH], F32, tag="fc")
                for k in range(KC):
                    nc.tensor.matmul(
                        fps[:rows, :], H2A[:, k, ds(s0, slen), :], fcw[:, k, :],
                        start=(k == 0), stop=False,
                    )
                nc.tensor.matmul(
                    fps[:rows, :], ones[:, :rows], fcbsb[:, n, :],
                    start=False, stop=True,
                )
                fco = wk.tile([128, VCH], F32, tag="pf")
                nc.vector.tensor_copy(out=fco[:rows, :], in_=fps[:rows, :])
                nc.sync.dma_start(
                    outd[ds(s0 * BC, rows), ds(n * VCH, VCH)], fco[:rows, :]
                )

    nc.compile()
    return nc


def st_alias(H2A, t):
    return H2A[:, :, t, :]


_PROG_CACHE = {}


def _get_prog(ns):
    if ns not in _PROG_CACHE:
        _PROG_CACHE[ns] = build_program(ns)
    return _PROG_CACHE[ns]


def prepare_inputs(spatial_feature, global_image, encoded_captions, emb,
                   w_ih1, w_hh1, b_ih1, b_hh1, s_wx, s_bx, s_wh, s_bh,
                   w_ih2, w_hh2, b_ih2, b_hh2, aff_s_w, aff_s_b, aff_h_w,
                   aff_h_b, ws_w, ws_b, wg_w, wg_b, wv_w, wv_b, wh_w, wh_b,
                   wp_w, wp_b, fc_w, fc_b, ns):
    """Host-side sharding / layout prep. Returns per-core input maps."""
    NR = ns * BC
    NJ = (NR + 127) // 128
    shared = {
        "emb": np.asarray(emb, dtype=bfnp),
        "W1xT": _tile_w(np.asarray(w_ih1)[:, D:].T),
        "WsxT": _tile_w(np.asarray(s_wx)[:, D:].T),
        "WvT": _tile_w(np.asarray(wv_w).T),
        "U1T": _tile_w(np.asarray(w_ih1)[:, :D].T),
        "Whh1T": _tile_w(np.asarray(w_hh1).T),
        "UsT": _tile_w(np.asarray(s_wx)[:, :D].T),
        "SwhT": _tile_w(np.asarray(s_wh).T),
        "AffST": _tile_w(np.asarray(aff_s_w).T),
        "AffHT": _tile_w(np.asarray(aff_h_w).T),
        "WgT": _tile_w(np.asarray(wg_w).T),
        "WsT2": _tile_w(np.asarray(ws_w).T),
        "WpT": _tile_w(np.asarray(wp_w).T),
        "UaT": _tile_w(np.asarray(w_ih2)[:, :D].T),
        "Uh1T": _tile_w(np.asarray(w_ih2)[:, D:].T),
        "Whh2T": _tile_w(np.asarray(w_hh2).T),
        "FcT": np.ascontiguousarray(
            np.asarray(fc_w).T.reshape(KC, 128, NV, VCH).transpose(1, 0, 2, 3)
        ).astype(bfnp),
        "fcb": np.asarray(fc_b).reshape(1, NV, VCH).astype(bfnp),
        "whv": np.ascontiguousarray(
            np.asarray(wh_w).reshape(KC, 128).T
        ).astype(bfnp),
        "b1": _col_bias(np.asarray(b_ih1) + np.asarray(b_hh1)),
        "b2": _col_bias(np.asarray(b_ih2) + np.asarray(b_hh2)),
        "bs": _col_bias(np.asarray(s_bx) + np.asarray(s_bh)),
        "asb": _col_bias(np.asarray(aff_s_b)),
        "ahb": _col_bias(np.asarray(aff_h_b)),
        "wgb": _col_bias(np.asarray(wg_b)),
        "wsb": _col_bias(np.asarray(ws_b)),
        "wvb": _col_bias(np.asarray(wv_b)),
        "wpb": _col_bias(np.asarray(wp_b)),
    }
    toks = np.asarray(encoded_captions)[:, :ns].astype(np.int64)
    sp = np.asarray(spatial_feature, dtype=np.float32)
    gi = np.asarray(global_image, dtype=np.float32)

    in_maps = []
    for c in range(NCORES):
        rows = slice(c * BC, (c + 1) * BC)
        # gather indices, t-major rows (t*BC + b), padded with 0
        tm = toks[rows].T.reshape(-1)  # [ns*BC]
        idx = np.zeros(NJ * 128, dtype=np.int32)
        idx[: tm.shape[0]] = tm.astype(np.int32)
        idx = np.ascontiguousarray(idx.reshape(NJ, 128).T)  # [128, NJ]
        spT = sp[rows].reshape(BC, P, D).transpose(2, 0, 1)  # [D, BC, P]
        spT = np.ascontiguousarray(
            spT.reshape(KC, 128, BC, P).transpose(1, 0, 2, 3)
        ).astype(bfnp)
        giT = gi[rows].T  # [D, BC]
        giT = np.ascontiguousarray(
            giT.reshape(KC, 128, BC).transpose(1, 0, 2)
        ).astype(bfnp)
        im = dict(shared)
        im.update({"idx": idx, "spT": spT, "giT": giT})
        in_maps.append(im)
    return in_maps


def kernel(**inputs) -> np.ndarray:
    ns = int(os.environ.get("KLSTM_NS", NS_FULL))
    caption_lengths = inputs.pop("caption_lengths", None)  # unused (all == T)
    del caption_lengths
    in_maps = prepare_inputs(ns=ns, **inputs)
    nc = _get_prog(ns)
    res = run_bass_kernel_spmd(nc, in_maps, list(range(NCORES)))
    out = np.empty((B, ns, V), dtype=np.float32)
    for c in range(NCORES):
        o = res.results[c]["out"].reshape(ns, BC, V)
        out[c * BC : (c + 1) * BC] = o.transpose(1, 0, 2)
    return out
